# revision 3
# baseline (speedup 1.0000x reference)
"""MoCo hard-example-mining loss (topk_masking) on 8 Trainium2 NeuronCores.

Strategy (sharding_hint: shard queue along K):
  The reference computes dist = euclid(feat_q, queue_eff.T) [N=512, K=65536],
  then masked max (hard positive) / min (hard negative) per row, then a
  scalar soft-margin loss.  After the enqueue step, queue_eff columns are:
    - cols [0, 512):  feat_k.T with labels = targets   (the "special" block)
    - cols [512, 64K): original L2-normalized queue columns, labels = 0
  For the zero-label region the mask is row-constant and ||z_j||^2 == 1, so
  per row only an extreme of p_ij = <feat_q_i, z_j> over that region is
  needed — and only ONE side per row:
    - rows with target != 0: the region holds only negatives -> need max_j p
    - rows with target == 0: the region holds only positives -> need min_j p
  Sign-flipping the target==0 rows of feat_q on the host turns both cases
  into a single per-row MAX on device (min_j p = -max_j <-q, z>).  The
  512-column special block and the final scalar loss are computed exactly on
  the host in float64 (trivial cost).

  Device (per core, fp8e4 + DoubleRow): the 65024 zero-label columns are
  padded to 65536 with duplicate columns (harmless for max) and sharded 8192
  per core.  128 DoubleRow matmuls (256-deep contraction, 512-wide moving)
  fill [128, 1024] 2-bank PSUM pairs; one DVE max tensor_reduce per pair
  accumulates into a [128, 4, 8] slot tile; a final tiny reduce + 2 KB DMA
  returns [128, 4] row maxima (row index = m*128 + p).  Host reduces across
  cores.  fp8e4 noise on p (~0.05 abs on extremes ~100) is far inside the
  2e-2 loss tolerance.
"""

import sys
import types
import numpy as np
import ml_dtypes

N, DIM, K, B = 512, 512, 65536, 512
NCORES = 8
KZ = K - B            # zero-label columns
CPC = K // NCORES     # padded columns per core (8192)
NT = CPC // 512       # 512-wide column tiles per core (16)
BIG = 9999999.0

LAST_RESULTS = None   # BassKernelResults of the most recent device run
_NC_CACHE = {}


def _install_axon_hooks_shim():
    """antenv.axon_hooks is absent on this image; bass_utils imports it when
    NTFF tracing is requested.  Provide the tiny get/set module and register
    the ctypes-based NTFF hook so trace=True / BASS_TRACE=1 works."""
    try:
        import antenv  # noqa: F401
    except ImportError:
        return
    if "antenv.axon_hooks" in sys.modules:
        return
    mod = types.ModuleType("antenv.axon_hooks")
    mod._hook = None

    def set_axon_ntff_profile_hook(h):
        mod._hook = h

    def get_axon_ntff_profile_hook():
        return mod._hook

    mod.set_axon_ntff_profile_hook = set_axon_ntff_profile_hook
    mod.get_axon_ntff_profile_hook = get_axon_ntff_profile_hook
    sys.modules["antenv.axon_hooks"] = mod
    sys.modules["antenv"].axon_hooks = mod
    try:
        from trn_agent_boot.trn_boot import _ntff_profile_via_ctypes

        mod._hook = _ntff_profile_via_ctypes("/opt/axon/libaxon_pjrt.so")
    except Exception:
        pass


def _build_nc():
    """Build + compile the per-core Bass program (identical on all cores)."""
    import concourse.bacc as bacc
    import concourse.mybir as mybir
    from concourse.tile import TileContext

    f8 = mybir.dt.float8e4
    f32 = mybir.dt.float32
    DR = mybir.MatmulPerfMode.DoubleRow

    nc = bacc.Bacc("TRN2", debug=False, target_bir_lowering=False)
    # qT8[p, kk, i] = feat_q'[i, kk*128+p]  (sign-flipped rows for target==0)
    qT = nc.dram_tensor("qT8", [128, 4, N], f8, kind="ExternalInput")
    # slab8[p, 4*n+kk, c] = z[kk*128+p, n*512+c]  (per-core column slab)
    slab = nc.dram_tensor("slab8", [128, 4 * NT, 512], f8, kind="ExternalInput")
    # o[p, m] = max_j p'_ij for row i = m*128+p
    o = nc.dram_tensor("o", [128, 4], f32, kind="ExternalOutput")

    bf16 = mybir.dt.bfloat16
    # PSUM pairs (32 total) whose evacuation goes straight to a DVE fp32
    # reduce; the rest are copied psum->bf16 by ScalarE and max-reduced on
    # DVE at the 16-bit 2x rate, balancing the two engines (~26.5 us each)
    DIRECT = {2, 6, 9, 13, 16, 20, 23, 27, 31}

    with TileContext(nc) as tc:
        with (
            tc.tile_pool(name="qpool", bufs=1) as qpool,
            tc.tile_pool(name="spool", bufs=8) as spool,
            tc.tile_pool(name="btpool", bufs=4) as btpool,
            tc.tile_pool(name="opool", bufs=1) as opool,
            tc.tile_pool(name="pspool", bufs=4, space="PSUM") as pspool,
        ):
            # HAM warmup: tiny matmuls during the preamble/DMA fill so the PE
            # clock gate is at 8/8 (2.4 GHz) when the real stream starts.
            # Kept short enough that the warmup LDW/MM pairs don't fill the
            # 64-deep PE queue and delay the first real matmul.
            warm = qpool.tile([128, 16], f8, name="warm")
            nc.gpsimd.memset(warm, 0.0)
            wps = pspool.tile([128, 1024], f32, name="wps", tag="ps")
            for _ in range(28):
                nc.tensor.matmul(wps[0:16, 0:16], warm, warm)

            # stage qT (256 KB) and the first slab tile
            qt = qpool.tile([128, 4, N], f8, name="qt")
            nc.sync.dma_start(out=qt, in_=qT.ap())
            st0 = spool.tile([128, 4, 512], f8, name="st", tag="st")
            nc.sync.dma_start(out=st0, in_=slab.ap()[:, 0:4, :])

            # osb[p, m, pair] accumulates the per-pair maxima (bf16 so the
            # 16-bit DVE fast path applies; ~0.4 abs noise on |p|~100 is
            # irrelevant at the 2e-2 loss tolerance)
            osb = opool.tile([128, 4, 8], bf16, name="osb")
            of = opool.tile([128, 4], f32, name="of")

            for pair in range(NT // 2):
                sts = []
                for half in range(2):
                    n = pair * 2 + half
                    if n == 0:
                        st = st0
                    else:
                        st = spool.tile([128, 4, 512], f8, name="st", tag="st")
                        nc.sync.dma_start(
                            out=st, in_=slab.ap()[:, 4 * n : 4 * n + 4, :]
                        )
                    sts.append(st)
                for m in range(4):
                    j = pair * 4 + m
                    ps = pspool.tile([128, 1024], f32, name="ps", tag="ps")
                    for half in range(2):
                        for kp in range(2):
                            nc.tensor.matmul(
                                ps[:, half * 512 : (half + 1) * 512],
                                qt[:, 2 * kp : 2 * kp + 2, m * 128 : (m + 1) * 128],
                                sts[half][:, 2 * kp : 2 * kp + 2, :],
                                start=(kp == 0),
                                stop=(kp == 1),
                                perf_mode=DR,
                            )
                    if j in DIRECT:
                        nc.vector.tensor_reduce(
                            osb[:, m, pair : pair + 1], ps,
                            axis=mybir.AxisListType.X, op=mybir.AluOpType.max,
                        )
                    else:
                        bt = btpool.tile([128, 1024], bf16, name="bt", tag="bt")
                        nc.scalar.copy(bt, ps)
                        nc.vector.tensor_reduce(
                            osb[:, m, pair : pair + 1], bt,
                            axis=mybir.AxisListType.X, op=mybir.AluOpType.max,
                        )

            nc.vector.tensor_reduce(
                of, osb, axis=mybir.AxisListType.X, op=mybir.AluOpType.max
            )
            nc.sync.dma_start(out=o.ap(), in_=of)

    nc.compile()
    return nc


def _get_nc():
    if "nc" not in _NC_CACHE:
        _install_axon_hooks_shim()
        _NC_CACHE["nc"] = _build_nc()
    return _NC_CACHE["nc"]


def _host_reference(feat_q, feat_k, targets, queue, queue_label):
    """Exact numpy fallback (float64) — used only if input assumptions
    (zero labels / normalized columns outside the enqueue block) fail."""
    fq = feat_q.astype(np.float64)
    fk = feat_k.astype(np.float64)
    t = targets.astype(np.int64)
    q = queue.astype(np.float64).copy()
    ql = queue_label.astype(np.int64).copy()
    q[:, : fk.shape[0]] = fk.T
    ql[: fk.shape[0]] = t
    xx = (fq * fq).sum(1)[:, None]
    yy = (q * q).sum(0)[None, :]
    sq = xx + yy - 2.0 * (fq @ q)
    dist = np.sqrt(np.clip(sq, 1e-12, None))
    is_pos = t[:, None] == ql[None, :]
    dist_ap = np.max(dist - BIG * (~is_pos), axis=1)
    dist_an = np.min(dist + BIG * is_pos, axis=1)
    return _loss(dist_ap, dist_an)


def _loss(dist_ap, dist_an):
    diff = dist_an - dist_ap
    loss_soft = np.mean(np.logaddexp(0.0, -diff))
    if np.isinf(loss_soft):
        return np.float32(np.mean(np.maximum(dist_ap - dist_an + 0.3, 0.0)))
    return np.float32(loss_soft)


def kernel(feat_q, feat_k, targets, queue, queue_label):
    feat_q = np.asarray(feat_q, dtype=np.float32)
    feat_k = np.asarray(feat_k, dtype=np.float32)
    targets = np.asarray(targets)
    queue = np.asarray(queue, dtype=np.float32)
    queue_label = np.asarray(queue_label)

    t = targets.astype(np.int64)
    Z = queue[:, B:]  # zero-label region, untouched by the enqueue

    # Guards for the two structural assumptions this split relies on.
    ok = not np.any(queue_label != 0)
    if ok:
        sample = np.linspace(0, KZ - 1, 512, dtype=np.int64)
        yy_s = np.einsum("ij,ij->j", Z[:, sample], Z[:, sample], dtype=np.float64)
        ok = bool(np.max(np.abs(yy_s - 1.0)) < 1e-3)
    if not ok:
        return _host_reference(feat_q, feat_k, targets, queue, queue_label)

    # ---- device part: per-row max of feat_q' @ Z over the zero-label region
    fp8 = ml_dtypes.float8_e4m3
    sign = np.where(t == 0, -1.0, 1.0).astype(np.float32)
    fq8 = (feat_q * sign[:, None]).astype(fp8)          # [N, dim]
    qtd = np.ascontiguousarray(fq8.T.reshape(4, 128, N).transpose(1, 0, 2))
    Z8 = Z.astype(fp8)                                   # [dim, KZ]
    in_maps = []
    for c in range(NCORES):
        lo = c * CPC
        hi = min((c + 1) * CPC, KZ)
        sl = np.empty((DIM, CPC), dtype=fp8)
        sl[:, : hi - lo] = Z8[:, lo:hi]
        if hi - lo < CPC:  # pad the tail core with duplicate columns
            sl[:, hi - lo :] = Z8[:, : CPC - (hi - lo)]
        sld = np.ascontiguousarray(
            sl.reshape(4, 128, NT, 512).transpose(1, 2, 0, 3).reshape(128, 4 * NT, 512)
        )
        in_maps.append({"qT8": qtd, "slab8": sld})

    from concourse import bass_utils

    nc = _get_nc()
    res = bass_utils.run_bass_kernel_spmd(nc, in_maps, core_ids=list(range(NCORES)))
    global LAST_RESULTS
    LAST_RESULTS = res

    pmx = np.full(N, -np.inf)
    for c in range(NCORES):
        oc = np.asarray(res.results[c]["o"], dtype=np.float64)  # [128, 4]
        pmx = np.maximum(pmx, oc.T.reshape(N))  # row (m*128+p) <- [p, m]

    # ---- host part: special 512-column block, exact in float64
    fq = feat_q.astype(np.float64)
    fk = feat_k.astype(np.float64)
    xx = (fq * fq).sum(1)
    kk_ = (fk * fk).sum(1)
    G = fq @ fk.T
    sqB = xx[:, None] + kk_[None, :] - 2.0 * G
    distB = np.sqrt(np.clip(sqB, 1e-12, None))
    maskB = t[:, None] == t[None, :]
    apB = np.max(distB - BIG * (~maskB), axis=1)
    anB = np.min(distB + BIG * maskB, axis=1)

    # zero-label region: ||z_j||^2 == 1; for t!=0 rows pmx = max_j p (hard
    # negative via min dist); for t==0 rows pmx = -min_j p (hard positive
    # via max dist)
    tz = t == 0
    an_z = np.where(
        tz, BIG, np.sqrt(np.clip(xx + 1.0 - 2.0 * pmx, 1e-12, None))
    )
    ap_z = np.where(
        tz, np.sqrt(np.clip(xx + 1.0 + 2.0 * pmx, 1e-12, None)), -BIG
    )

    dist_ap = np.maximum(apB, ap_z)
    dist_an = np.minimum(anB, an_z)
    return _loss(dist_ap, dist_an)
